# revision 3
# baseline (speedup 1.0000x reference)
"""Trainium2 Bass kernel v3 for nn_Complex2LayerMAPGraphConvolution.

Key change vs v1: dma_gather descriptor generation (the serial bottleneck,
~8.4ns/row on one SWDGE queue) is spread over 4 SWDGE queues -> ~2.7ns/row,
with per-(block,range)-segment gather calls and a deep gather pool so the
Pool engine is never stalled by tile-reuse dependencies.
"""

import os
import sys

for _p in ("/opt/trn_rl_repo", "/root/.axon_site/_ro/trn_rl_repo"):
    if os.path.isdir(_p) and _p not in sys.path:
        sys.path.insert(0, _p)

import numpy as np

import concourse.bass as bass
import concourse.tile as tile
from concourse import mybir, bacc
from concourse.masks import make_identity

P = 128
F16 = mybir.dt.float16
F32 = mybir.dt.float32
I16 = mybir.dt.int16

NQ = 4


class Cfg:
    def __init__(self, n_nodes, n_edges, cores=8, sg=3, rsz=25000):
        assert n_nodes % cores == 0
        self.N = n_nodes
        self.E = n_edges
        self.CORES = cores
        self.NPC = n_nodes // cores            # nodes per core
        self.NB = (self.NPC + P - 1) // P      # dest blocks per core
        self.NV_LAST = self.NPC - (self.NB - 1) * P
        self.SG = sg                           # blocks per supergroup
        self.RSZ = min(rsz, n_nodes)           # rows per index range
        self.NR = (n_nodes + self.RSZ - 1) // self.RSZ
        assert self.RSZ <= 32767


def host_prep(cfg, real, imag, ew, q, ent, ccf, W1, b1, W2, b2, W3, b3,
              row, col):
    """Pure index/layout preprocessing (sharding) + weight layout prep."""
    N, E, C, NPC, NB = cfg.N, cfg.E, cfg.CORES, cfg.NPC, cfg.NB
    NR, RSZ, SG = cfg.NR, cfg.RSZ, cfg.SG

    core = row // NPC
    r_local = row - core * NPC
    blk = r_local // P
    lrow = (r_local - blk * P).astype(np.float32)
    rid = col // RSZ

    # per-(core, block, range) edge counts; chunk counts equalized over cores
    cnt = np.zeros((C, NB, NR), np.int64)
    np.add.at(cnt, (core, blk, rid), 1)
    seg_cpb = np.maximum(-(-cnt.max(axis=0) // P), 1)   # [NB, NR] chunks

    # segment (gather call) order: for supergroup g: for r: for b in g
    seg_start = np.zeros((NB, NR), np.int64)
    calls = []          # (start_chunk, n_chunks, range_id, block)
    nch = 0
    n_groups = (NB + SG - 1) // SG
    for g in range(n_groups):
        bs = list(range(g * SG, min((g + 1) * SG, NB)))
        for r in range(NR):
            for b in bs:
                w = int(seg_cpb[b, r])
                seg_start[b, r] = nch
                calls.append((nch, w, r, b))
                nch += w
    NCH = nch

    # edge -> (core, chunk, partition); pad slots point at range row 0
    key = (core.astype(np.int64) * NB + blk) * NR + rid
    order = np.argsort(key, kind="stable")
    ks = key[order]
    starts = np.searchsorted(ks, np.arange(C * NB * NR))
    rank = np.arange(E) - starts[ks]
    c_ = ks // (NB * NR)
    b_ = (ks // NR) % NB
    r_ = ks % NR
    chunk = seg_start[b_, r_] + rank // P
    part = rank % P
    e = order

    lrA = np.zeros((C, P, NCH), np.float32)
    edatA = np.zeros((C, P, 3, NCH), np.float32)
    lrA[c_, part, chunk] = lrow[e]
    edatA[c_, part, 0, chunk] = ent[e]
    edatA[c_, part, 1, chunk] = ccf[e]
    edatA[c_, part, 2, chunk] = ew[e]

    # int16 gather indices (pads -> row 0 of the range, weight 0)
    gidxA = np.zeros((C, 16, NCH * 8), np.int16)
    pos = chunk * P + part
    gidxA[c_, pos % 16, pos // 16] = (col[e] - r_ * RSZ).astype(np.int16)
    gidxA = np.tile(gidxA, (1, 8, 1))                  # [C, 128, NCH*8]

    tab = np.concatenate([real, imag], axis=1).astype(np.float16)  # [N, 128]

    def stk_a(W):
        H, Fd = W.shape
        out = np.zeros((2 * Fd, 2 * H), np.float16)
        out[:Fd, :H] = W.T
        out[Fd:, H:] = W.T
        return out

    def stk_b(W):
        H, Fd = W.shape
        out = np.zeros((2 * Fd, 2 * H), np.float16)
        out[Fd:, :H] = -W.T
        out[:Fd, H:] = W.T
        return out

    def brow(b):
        out = np.zeros((2 * len(b), 1), np.float32)
        out[len(b):, 0] = 2.0 * b
        return out

    consts = {
        "qcol": np.full((P, 1), np.float32(q), np.float32),
        "wa1": stk_a(W1), "wb1": stk_b(W1), "brow1": brow(b1),
        "wa2": stk_a(W2), "wb2": stk_b(W2), "brow2": brow(b2),
        "w3s": W3.T.astype(np.float16).copy(),           # [2H, O]
        "b3col": b3.astype(np.float32).reshape(-1, 1).copy(),
    }
    in_maps = []
    for c in range(cfg.CORES):
        m = {"table1": tab, "gidx": gidxA[c], "lr": lrA[c], "edat": edatA[c]}
        m.update(consts)
        in_maps.append(m)
    meta = {"NCH": NCH, "calls": calls, "n_groups": n_groups,
            "seg_cpb": seg_cpb, "WMAX": int(seg_cpb.max())}
    return in_maps, meta


def build_nc(cfg, meta):
    N, NPC, NB, SG, NR, RSZ = (cfg.N, cfg.NPC, cfg.NB, cfg.SG,
                               cfg.NR, cfg.RSZ)
    NCH = meta["NCH"]
    calls = meta["calls"]
    n_groups = meta["n_groups"]
    WMAX = meta["WMAX"]
    O = 16
    GB = int(os.environ.get("GNN_GB", "32"))       # gather pool depth
    ACTF = int(os.environ.get("GNN_ACTF", "2"))    # 1/ACTF chunks on ACT
    nc = bacc.Bacc(num_devices=cfg.CORES, num_swdge_queues=NQ)

    tab1 = nc.declare_dram_parameter("table1", [N, P], F16, isOutput=False)
    gidx_d = nc.declare_dram_parameter("gidx", [P, NCH * 8], I16, isOutput=False)
    lr_d = nc.declare_dram_parameter("lr", [P, NCH], F32, isOutput=False)
    edat_d = nc.declare_dram_parameter("edat", [P, 3, NCH], F32, isOutput=False)
    qcol_d = nc.declare_dram_parameter("qcol", [P, 1], F32, isOutput=False)
    wa_d = [nc.declare_dram_parameter("wa1", [P, P], F16, isOutput=False),
            nc.declare_dram_parameter("wa2", [P, P], F16, isOutput=False)]
    wb_d = [nc.declare_dram_parameter("wb1", [P, P], F16, isOutput=False),
            nc.declare_dram_parameter("wb2", [P, P], F16, isOutput=False)]
    brow_d = [nc.declare_dram_parameter("brow1", [P, 1], F32, isOutput=False),
              nc.declare_dram_parameter("brow2", [P, 1], F32, isOutput=False)]
    w3s_d = nc.declare_dram_parameter("w3s", [P, O], F16, isOutput=False)
    b3_d = nc.declare_dram_parameter("b3col", [O, 1], F32, isOutput=False)
    out_t = nc.declare_dram_parameter("out_t", [O, NPC], F32, isOutput=True)

    tab2in = nc.dram_tensor("tab2in", [NPC, P], F16)
    tab2f = nc.dram_tensor("tab2f", [N, P], F16, addr_space="Shared")
    # per-chunk scatter masks are identical in both layers (same edges and
    # edge weights): build on DVE in layer 1, spill to DRAM, reload in layer 2
    masks_d = nc.dram_tensor("masks_spill", [P, NCH, 256], F16)

    AluOp = mybir.AluOpType
    Act = mybir.ActivationFunctionType

    with tile.TileContext(nc) as tc:
        import contextlib
        with contextlib.ExitStack() as ctx:
            singles = ctx.enter_context(tc.tile_pool(name="singles", bufs=1))
            prep = ctx.enter_context(tc.tile_pool(name="prep", bufs=1))
            gpool = ctx.enter_context(tc.tile_pool(name="gpool", bufs=GB))
            mpool = ctx.enter_context(tc.tile_pool(name="mpool", bufs=int(os.environ.get("GNN_MB", "16"))))
            p2pool = ctx.enter_context(tc.tile_pool(name="p2pool", bufs=2))
            lopool = ctx.enter_context(tc.tile_pool(name="lopool", bufs=2))
            twpool = ctx.enter_context(tc.tile_pool(name="twpool", bufs=2))
            topool = ctx.enter_context(tc.tile_pool(name="topool", bufs=2))
            pp_s = ctx.enter_context(tc.tile_pool(name="pp_s", bufs=int(os.environ.get("GNN_PS", "4")), space="PSUM"))
            pp_l = ctx.enter_context(tc.tile_pool(name="pp_l", bufs=int(os.environ.get("GNN_PL", "2")), space="PSUM"))
            pp_x = ctx.enter_context(tc.tile_pool(name="pp_x", bufs=int(os.environ.get("GNN_PX", "2")), space="PSUM"))

            # ---- resident metadata + constants ----
            gidx_s = singles.tile([P, NCH * 8], I16)
            lr_s = singles.tile([P, NCH], F32)
            wr_s = singles.tile([P, NCH], F32)
            wi_s = singles.tile([P, NCH], F32)
            nc.sync.dma_start(out=gidx_s, in_=gidx_d[:, :])
            nc.sync.dma_start(out=lr_s, in_=lr_d[:, :])

            qcol = singles.tile([P, 1], F32)
            nc.sync.dma_start(out=qcol, in_=qcol_d[:, :])
            wa = [singles.tile([P, P], F16, name=f"wa{i}") for i in range(2)]
            wb = [singles.tile([P, P], F16, name=f"wb{i}") for i in range(2)]
            brow = [singles.tile([P, 1], F32, name=f"brow{i}") for i in range(2)]
            for i in range(2):
                nc.sync.dma_start(out=wa[i], in_=wa_d[i][:, :])
                nc.sync.dma_start(out=wb[i], in_=wb_d[i][:, :])
                nc.sync.dma_start(out=brow[i], in_=brow_d[i][:, :])
            w3s = singles.tile([P, O], F16)
            nc.sync.dma_start(out=w3s, in_=w3s_d[:, :])
            b3c = singles.tile([O, 1], F32)
            nc.sync.dma_start(out=b3c, in_=b3_d[:, :])

            iota = singles.tile([P, P], F16)
            nc.gpsimd.iota(iota, pattern=[[1, P]], base=0, channel_multiplier=0,
                           allow_small_or_imprecise_dtypes=True)
            ident = singles.tile([P, P], F16)
            make_identity(nc, ident)

            # gather tiles: fixed set, memset once (pad slots may stay stale)
            g_tiles = [singles.tile([P, WMAX, P], F16, name=f"gt{i}")
                       for i in range(GB)]
            for gt in g_tiles:
                nc.vector.memset(gt, 0.0)
            widths = sorted({w for _, w, _, _ in calls})
            nidx_regs = {}
            if os.environ.get("GNN_REG", "1") == "1":
                for w in widths:
                    reg = ctx.enter_context(nc.gpsimd.register(name=f"nidx{w}"))
                    nc.gpsimd.reg_mov(reg, w * P)
                    nidx_regs[w] = reg

            # ---- edge weight prep: wr = ew*cos(q*(ent+ccf)), wi = ew*sin ----
            edat_s = prep.tile([P, 3, NCH], F32)
            nc.sync.dma_start(out=edat_s, in_=edat_d[:, :, :])
            phase = prep.tile([P, NCH], F32)
            nc.vector.tensor_tensor(out=phase, in0=edat_s[:, 0, :],
                                    in1=edat_s[:, 1, :], op=AluOp.add)
            nc.vector.tensor_scalar(out=phase, in0=phase, scalar1=qcol[:, 0:1],
                                    scalar2=None, op0=AluOp.mult)
            pio2 = singles.tile([P, 1], F32)
            nc.vector.memset(pio2, float(np.pi / 2))
            trig = prep.tile([P, NCH], F32)
            nc.scalar.activation(out=trig, in_=phase, func=Act.Sin,
                                 bias=pio2[:, 0:1], scale=-1.0)
            nc.vector.tensor_tensor(out=wr_s, in0=edat_s[:, 2, :], in1=trig,
                                    op=AluOp.mult)
            nc.scalar.activation(out=trig, in_=phase, func=Act.Sin)
            nc.vector.tensor_tensor(out=wi_s, in0=edat_s[:, 2, :], in1=trig,
                                    op=AluOp.mult)

            # ---- two graph-conv layers ----
            call_no = 0
            for L in range(2):
                tab_h = tab1 if L == 0 else tab2f
                for g in range(n_groups):
                    bs = list(range(g * SG, min((g + 1) * SG, NB)))
                    my_calls = [cl for cl in calls if cl[3] in bs]
                    # one PSUM bank per block in the supergroup
                    pair = {}
                    for k, b in enumerate(bs):
                        pair[b] = pp_s.tile([P, 256], F32, space="PSUM",
                                            tag="ps", name=f"ps{L}_{g}_{k}")
                    first_c = {b: None for b in bs}
                    last_c = {}
                    for (c0, w, r, b) in my_calls:
                        if first_c[b] is None:
                            first_c[b] = c0
                        last_c[b] = c0 + w - 1
                    for (c0, w, r, b) in my_calls:
                        ci = call_no % NQ
                        gt = g_tiles[call_no % GB]
                        call_no += 1
                        nc.gpsimd.dma_gather(
                            out_ap=gt[:, :w, :],
                            in_ap=tab_h[r * RSZ:, :],
                            idxs_ap=gidx_s[:, c0 * 8:(c0 + w) * 8],
                            num_idxs=w * P, num_idxs_reg=nidx_regs.get(w, w * P),
                            elem_size=P, queue_num=ci)
                        psum = pair[b]
                        mt = mpool.tile([P, WMAX, 256], F16, tag="m",
                                        name=f"m{L}_{c0}")
                        if L == 1:
                            nc.sync.dma_start(out=mt[:, :w, :],
                                              in_=masks_d[:, c0:c0 + w, :])
                        for j in range(w):
                            c = c0 + j
                            if L == 0:
                                if c % ACTF == ACTF - 1:
                                    eqm = mpool.tile([P, P], F16, tag="eq",
                                                     name=f"eq{L}_{c}")
                                    nc.vector.tensor_scalar(
                                        out=eqm, in0=iota[:, :],
                                        scalar1=lr_s[:, c:c + 1], scalar2=None,
                                        op0=AluOp.is_equal)
                                    nc.scalar.mul(mt[:, j, 0:P], eqm,
                                                  wr_s[:, c:c + 1])
                                    nc.scalar.mul(mt[:, j, P:256], eqm,
                                                  wi_s[:, c:c + 1])
                                else:
                                    nc.vector.tensor_scalar(
                                        out=mt[:, j, 0:P], in0=iota[:, :],
                                        scalar1=lr_s[:, c:c + 1],
                                        scalar2=wr_s[:, c:c + 1],
                                        op0=AluOp.is_equal, op1=AluOp.mult)
                                    nc.vector.tensor_scalar(
                                        out=mt[:, j, P:256], in0=iota[:, :],
                                        scalar1=lr_s[:, c:c + 1],
                                        scalar2=wi_s[:, c:c + 1],
                                        op0=AluOp.is_equal, op1=AluOp.mult)
                            nc.tensor.matmul(
                                psum[:, :],
                                lhsT=gt[:, j, :], rhs=mt[:, j, :],
                                start=(c == first_c[b]), stop=(c == last_c[b]),
                                skip_group_check=True)
                        if L == 0:
                            nc.sync.dma_start(out=masks_d[:, c0:c0 + w, :],
                                              in_=mt[:, :w, :])
                    # finalize blocks of this supergroup
                    for b in bs:
                        psum = pair[b]
                        p2c = p2pool.tile([P, 256], F16, tag="p2",
                                          name=f"p2_{L}_{b}")
                        nc.scalar.activation(out=p2c, in_=psum[:, :],
                                             func=Act.Copy)
                        psl = pp_l.tile([P, P], F32, space="PSUM", tag="pl",
                                        name=f"pl{L}_{b}")
                        nc.tensor.matmul(psl[:, :], lhsT=wa[L], rhs=p2c[:, 0:P],
                                         start=True, stop=False)
                        nc.tensor.matmul(psl[:, :], lhsT=wb[L],
                                         rhs=p2c[:, P:256],
                                         start=False, stop=True)
                        lout = lopool.tile([P, P], F16, tag="lo",
                                           name=f"lo{L}_{b}")
                        nc.scalar.activation(out=lout, in_=psl, func=Act.Relu,
                                             bias=brow[L][:, 0:1])
                        nv = P if b < NB - 1 else cfg.NV_LAST
                        if L == 0:
                            pst = pp_x.tile([P, P], F16, space="PSUM",
                                            tag="px", name=f"px{b}")
                            nc.tensor.transpose(pst[:, :], lout[:, :],
                                                ident[:, :])
                            tblw = twpool.tile([P, P], F16, tag="tw",
                                               name=f"tw{b}")
                            nc.vector.tensor_copy(out=tblw, in_=pst)
                            nc.sync.dma_start(
                                out=tab2in[b * P:b * P + nv, :],
                                in_=tblw[:nv, :])
                        else:
                            pso = pp_x.tile([P, P], F32, space="PSUM",
                                            tag="px", name=f"pxo{b}")
                            nc.tensor.matmul(pso[:O, :], lhsT=w3s[:, :],
                                             rhs=lout[:, :], start=True,
                                             stop=True)
                            osb = topool.tile([O, P], F32, tag="to",
                                              name=f"to{b}")
                            nc.scalar.activation(out=osb, in_=pso[:O, :],
                                                 func=Act.Identity,
                                                 bias=b3c[:, 0:1])
                            nc.sync.dma_start(out=out_t[:, b * P:b * P + nv],
                                              in_=osb[:, :nv])
                if L == 0:
                    nc.gpsimd.collective_compute(
                        "AllGather", AluOp.bypass,
                        replica_groups=[list(range(cfg.CORES))],
                        ins=[tab2in.ap().opt()],
                        outs=[tab2f.ap().opt()],
                    )
    nc.compile()
    return nc


_CACHE = {}


def _get_nc(cfg, meta):
    key = (cfg.N, cfg.E, cfg.CORES, cfg.SG,
           tuple(c for call in meta["calls"] for c in call))
    if key not in _CACHE:
        _CACHE[key] = build_nc(cfg, meta)
    return _CACHE[key]


def run(cfg, inputs, trace=False):
    from concourse.bass_utils import run_bass_kernel_spmd

    in_maps, meta = host_prep(
        cfg,
        np.asarray(inputs["real_feature"], np.float32),
        np.asarray(inputs["imag_feature"], np.float32),
        np.asarray(inputs["edge_weight_sym"], np.float32),
        np.float32(inputs["exp_weight_q"]),
        np.asarray(inputs["edge_entropy"], np.float32),
        np.asarray(inputs["edge_cluster_coefficient"], np.float32),
        np.asarray(inputs["W1"], np.float32), np.asarray(inputs["b1"], np.float32),
        np.asarray(inputs["W2"], np.float32), np.asarray(inputs["b2"], np.float32),
        np.asarray(inputs["W3"], np.float32), np.asarray(inputs["b3"], np.float32),
        np.asarray(inputs["row"]).astype(np.int64),
        np.asarray(inputs["col"]).astype(np.int64),
    )
    nc = _get_nc(cfg, meta)
    res = run_bass_kernel_spmd(nc, in_maps, list(range(cfg.CORES)), trace=trace)
    out = np.empty((cfg.N, 16), np.float32)
    for c in range(cfg.CORES):
        out[c * cfg.NPC:(c + 1) * cfg.NPC, :] = res.results[c]["out_t"].T
    return out, res


def kernel(**inputs) -> np.ndarray:
    cfg = Cfg(100000, 1000000, cores=8)
    out, _ = run(cfg, inputs, trace=False)
    return out


# revision 4
# speedup vs baseline: 1.0080x; 1.0080x over previous
"""Trainium2 Bass kernel for nn_Complex2LayerMAPGraphConvolution (v3).

Complex-weighted 2-layer graph convolution + linear head on 8 NeuronCores,
edge-cut (destination-row-block) graph parallelism. ~1.6ms HW (vs 2.26ms v1).

Per core (owns N/8 = 12500 destination nodes):
  - edges bucketed by (dest 128-block, source range of 25000 rows); each
    (block, range) segment padded to whole 128-edge chunks, chunk counts
    equalized across cores (single SPMD program), pads point at range row 0
    with zero weight.
  - dma_gather pulls x[col] rows ([real|imag] f16, 256B) one per partition.
    Descriptor generation on the GPSIMD(Q7) complex is the serial bottleneck
    (~8.4ns/row per queue-pair): calls are one (block, range) segment each,
    round-robin over 4 SWDGE queues (disjoint Q7 core pairs) for ~2x overlap,
    with a deep gather-tile ring so the Pool engine rarely stalls on reuse.
  - per chunk the DVE builds a weighted one-hot scatter mask [Wr|Wi]
    ((iota==lrow)*w, fused is_equal+mult tensor_scalar; every 2nd chunk the
    two weight scalings go to the Scalar engine to balance load); TensorE
    computes G.T @ [Wr|Wi], accumulating all 4 complex spmm products in PSUM
    per destination block (supergroups of 3 blocks share PSUM banks).
  - masks are identical in both layers (same edges/weights): layer 1 spills
    them to DRAM on idle DMA queues; layer 2 reloads instead of rebuilding,
    freeing the Vector engine entirely in layer 2.
  - per block: FC + complex recombination folded into two stacked-weight
    matmuls; ReLU+bias on ScalarE; layer-1 output transposed to node-major
    f16 and AllGather'd so layer-2 gathers can read any source row.
  - layer 3 (linear head) fused per block off the layer-2 tile.
"""

import os
import sys

for _p in ("/opt/trn_rl_repo", "/root/.axon_site/_ro/trn_rl_repo"):
    if os.path.isdir(_p) and _p not in sys.path:
        sys.path.insert(0, _p)

import numpy as np

import concourse.bass as bass
import concourse.tile as tile
from concourse import mybir, bacc
from concourse.masks import make_identity

P = 128
F16 = mybir.dt.float16
F32 = mybir.dt.float32
I16 = mybir.dt.int16

NQ = 4


class Cfg:
    def __init__(self, n_nodes, n_edges, cores=8, sg=3, rsz=25000):
        assert n_nodes % cores == 0
        self.N = n_nodes
        self.E = n_edges
        self.CORES = cores
        self.NPC = n_nodes // cores            # nodes per core
        self.NB = (self.NPC + P - 1) // P      # dest blocks per core
        self.NV_LAST = self.NPC - (self.NB - 1) * P
        self.SG = sg                           # blocks per supergroup
        self.RSZ = min(rsz, n_nodes)           # rows per index range
        self.NR = (n_nodes + self.RSZ - 1) // self.RSZ
        assert self.RSZ <= 32767


def host_prep(cfg, real, imag, ew, q, ent, ccf, W1, b1, W2, b2, W3, b3,
              row, col):
    """Pure index/layout preprocessing (sharding) + weight layout prep."""
    N, E, C, NPC, NB = cfg.N, cfg.E, cfg.CORES, cfg.NPC, cfg.NB
    NR, RSZ, SG = cfg.NR, cfg.RSZ, cfg.SG

    core = row // NPC
    r_local = row - core * NPC
    blk = r_local // P
    lrow = (r_local - blk * P).astype(np.float32)
    rid = col // RSZ

    # per-(core, block, range) edge counts; chunk counts equalized over cores
    cnt = np.zeros((C, NB, NR), np.int64)
    np.add.at(cnt, (core, blk, rid), 1)
    seg_cpb = np.maximum(-(-cnt.max(axis=0) // P), 1)   # [NB, NR] chunks

    # segment (gather call) order: for supergroup g: for r: for b in g
    seg_start = np.zeros((NB, NR), np.int64)
    calls = []          # (start_chunk, n_chunks, range_id, block)
    nch = 0
    n_groups = (NB + SG - 1) // SG
    for g in range(n_groups):
        bs = list(range(g * SG, min((g + 1) * SG, NB)))
        for r in range(NR):
            for b in bs:
                w = int(seg_cpb[b, r])
                seg_start[b, r] = nch
                calls.append((nch, w, r, b))
                nch += w
    NCH = nch

    # edge -> (core, chunk, partition); pad slots point at range row 0
    key = (core.astype(np.int64) * NB + blk) * NR + rid
    order = np.argsort(key, kind="stable")
    ks = key[order]
    starts = np.searchsorted(ks, np.arange(C * NB * NR))
    rank = np.arange(E) - starts[ks]
    c_ = ks // (NB * NR)
    b_ = (ks // NR) % NB
    r_ = ks % NR
    chunk = seg_start[b_, r_] + rank // P
    part = rank % P
    e = order

    lrA = np.zeros((C, P, NCH), np.float32)
    edatA = np.zeros((C, P, 3, NCH), np.float32)
    lrA[c_, part, chunk] = lrow[e]
    edatA[c_, part, 0, chunk] = ent[e]
    edatA[c_, part, 1, chunk] = ccf[e]
    edatA[c_, part, 2, chunk] = ew[e]

    # int16 gather indices (pads -> row 0 of the range, weight 0)
    gidxA = np.zeros((C, 16, NCH * 8), np.int16)
    pos = chunk * P + part
    gidxA[c_, pos % 16, pos // 16] = (col[e] - r_ * RSZ).astype(np.int16)
    gidxA = np.tile(gidxA, (1, 8, 1))                  # [C, 128, NCH*8]

    tab = np.concatenate([real, imag], axis=1).astype(np.float16)  # [N, 128]

    def stk_a(W):
        H, Fd = W.shape
        out = np.zeros((2 * Fd, 2 * H), np.float16)
        out[:Fd, :H] = W.T
        out[Fd:, H:] = W.T
        return out

    def stk_b(W):
        H, Fd = W.shape
        out = np.zeros((2 * Fd, 2 * H), np.float16)
        out[Fd:, :H] = -W.T
        out[:Fd, H:] = W.T
        return out

    def brow(b):
        out = np.zeros((2 * len(b), 1), np.float32)
        out[len(b):, 0] = 2.0 * b
        return out

    consts = {
        "qcol": np.full((P, 1), np.float32(q), np.float32),
        "wa1": stk_a(W1), "wb1": stk_b(W1), "brow1": brow(b1),
        "wa2": stk_a(W2), "wb2": stk_b(W2), "brow2": brow(b2),
        "w3s": W3.T.astype(np.float16).copy(),           # [2H, O]
        "b3col": b3.astype(np.float32).reshape(-1, 1).copy(),
    }
    in_maps = []
    for c in range(cfg.CORES):
        m = {"table1": tab, "gidx": gidxA[c], "lr": lrA[c], "edat": edatA[c]}
        m.update(consts)
        in_maps.append(m)
    meta = {"NCH": NCH, "calls": calls, "n_groups": n_groups,
            "seg_cpb": seg_cpb, "WMAX": int(seg_cpb.max())}
    return in_maps, meta


def build_nc(cfg, meta):
    N, NPC, NB, SG, NR, RSZ = (cfg.N, cfg.NPC, cfg.NB, cfg.SG,
                               cfg.NR, cfg.RSZ)
    NCH = meta["NCH"]
    calls = meta["calls"]
    n_groups = meta["n_groups"]
    WMAX = meta["WMAX"]
    O = 16
    GB = int(os.environ.get("GNN_GB", "32"))       # gather pool depth
    ACTF = int(os.environ.get("GNN_ACTF", "2"))    # 1/ACTF chunks on ACT
    nc = bacc.Bacc(num_devices=cfg.CORES, num_swdge_queues=NQ)

    tab1 = nc.declare_dram_parameter("table1", [N, P], F16, isOutput=False)
    gidx_d = nc.declare_dram_parameter("gidx", [P, NCH * 8], I16, isOutput=False)
    lr_d = nc.declare_dram_parameter("lr", [P, NCH], F32, isOutput=False)
    edat_d = nc.declare_dram_parameter("edat", [P, 3, NCH], F32, isOutput=False)
    qcol_d = nc.declare_dram_parameter("qcol", [P, 1], F32, isOutput=False)
    wa_d = [nc.declare_dram_parameter("wa1", [P, P], F16, isOutput=False),
            nc.declare_dram_parameter("wa2", [P, P], F16, isOutput=False)]
    wb_d = [nc.declare_dram_parameter("wb1", [P, P], F16, isOutput=False),
            nc.declare_dram_parameter("wb2", [P, P], F16, isOutput=False)]
    brow_d = [nc.declare_dram_parameter("brow1", [P, 1], F32, isOutput=False),
              nc.declare_dram_parameter("brow2", [P, 1], F32, isOutput=False)]
    w3s_d = nc.declare_dram_parameter("w3s", [P, O], F16, isOutput=False)
    b3_d = nc.declare_dram_parameter("b3col", [O, 1], F32, isOutput=False)
    out_t = nc.declare_dram_parameter("out_t", [O, NPC], F32, isOutput=True)

    tab2in = nc.dram_tensor("tab2in", [NPC, P], F16)
    tab2f = nc.dram_tensor("tab2f", [N, P], F16, addr_space="Shared")
    # per-chunk scatter masks are identical in both layers (same edges and
    # edge weights): build on DVE in layer 1, spill to DRAM, reload in layer 2
    masks_d = nc.dram_tensor("masks_spill", [P, NCH, 256], F16)

    AluOp = mybir.AluOpType
    Act = mybir.ActivationFunctionType

    with tile.TileContext(nc) as tc:
        import contextlib
        with contextlib.ExitStack() as ctx:
            singles = ctx.enter_context(tc.tile_pool(name="singles", bufs=1))
            prep = ctx.enter_context(tc.tile_pool(name="prep", bufs=1))
            gpool = ctx.enter_context(tc.tile_pool(name="gpool", bufs=GB))
            mpool = ctx.enter_context(tc.tile_pool(name="mpool", bufs=int(os.environ.get("GNN_MB", "16"))))
            p2pool = ctx.enter_context(tc.tile_pool(name="p2pool", bufs=2))
            lopool = ctx.enter_context(tc.tile_pool(name="lopool", bufs=2))
            twpool = ctx.enter_context(tc.tile_pool(name="twpool", bufs=2))
            topool = ctx.enter_context(tc.tile_pool(name="topool", bufs=2))
            pp_s = ctx.enter_context(tc.tile_pool(name="pp_s", bufs=int(os.environ.get("GNN_PS", "4")), space="PSUM"))
            pp_l = ctx.enter_context(tc.tile_pool(name="pp_l", bufs=int(os.environ.get("GNN_PL", "2")), space="PSUM"))
            pp_x = ctx.enter_context(tc.tile_pool(name="pp_x", bufs=int(os.environ.get("GNN_PX", "2")), space="PSUM"))

            # ---- resident metadata + constants ----
            gidx_s = singles.tile([P, NCH * 8], I16)
            lr_s = singles.tile([P, NCH], F32)
            wr_s = singles.tile([P, NCH], F32)
            wi_s = singles.tile([P, NCH], F32)
            nc.sync.dma_start(out=gidx_s, in_=gidx_d[:, :])
            nc.sync.dma_start(out=lr_s, in_=lr_d[:, :])

            qcol = singles.tile([P, 1], F32)
            nc.sync.dma_start(out=qcol, in_=qcol_d[:, :])
            wa = [singles.tile([P, P], F16, name=f"wa{i}") for i in range(2)]
            wb = [singles.tile([P, P], F16, name=f"wb{i}") for i in range(2)]
            brow = [singles.tile([P, 1], F32, name=f"brow{i}") for i in range(2)]
            for i in range(2):
                nc.sync.dma_start(out=wa[i], in_=wa_d[i][:, :])
                nc.sync.dma_start(out=wb[i], in_=wb_d[i][:, :])
                nc.sync.dma_start(out=brow[i], in_=brow_d[i][:, :])
            w3s = singles.tile([P, O], F16)
            nc.sync.dma_start(out=w3s, in_=w3s_d[:, :])
            b3c = singles.tile([O, 1], F32)
            nc.sync.dma_start(out=b3c, in_=b3_d[:, :])

            iota = singles.tile([P, P], F16)
            nc.gpsimd.iota(iota, pattern=[[1, P]], base=0, channel_multiplier=0,
                           allow_small_or_imprecise_dtypes=True)
            ident = singles.tile([P, P], F16)
            make_identity(nc, ident)

            # gather tiles: fixed set, memset once (pad slots may stay stale)
            g_tiles = [singles.tile([P, WMAX, P], F16, name=f"gt{i}")
                       for i in range(GB)]
            for gt in g_tiles:
                nc.vector.memset(gt, 0.0)
            widths = sorted({w for _, w, _, _ in calls})
            nidx_regs = {}
            if os.environ.get("GNN_REG", "1") == "1":
                for w in widths:
                    reg = ctx.enter_context(nc.gpsimd.register(name=f"nidx{w}"))
                    nc.gpsimd.reg_mov(reg, w * P)
                    nidx_regs[w] = reg

            # ---- edge weight prep: wr = ew*cos(q*(ent+ccf)), wi = ew*sin ----
            edat_s = prep.tile([P, 3, NCH], F32)
            nc.sync.dma_start(out=edat_s, in_=edat_d[:, :, :])
            phase = prep.tile([P, NCH], F32)
            nc.vector.tensor_tensor(out=phase, in0=edat_s[:, 0, :],
                                    in1=edat_s[:, 1, :], op=AluOp.add)
            nc.vector.tensor_scalar(out=phase, in0=phase, scalar1=qcol[:, 0:1],
                                    scalar2=None, op0=AluOp.mult)
            pio2 = singles.tile([P, 1], F32)
            nc.vector.memset(pio2, float(np.pi / 2))
            trig = prep.tile([P, NCH], F32)
            nc.scalar.activation(out=trig, in_=phase, func=Act.Sin,
                                 bias=pio2[:, 0:1], scale=-1.0)
            nc.vector.tensor_tensor(out=wr_s, in0=edat_s[:, 2, :], in1=trig,
                                    op=AluOp.mult)
            nc.scalar.activation(out=trig, in_=phase, func=Act.Sin)
            nc.vector.tensor_tensor(out=wi_s, in0=edat_s[:, 2, :], in1=trig,
                                    op=AluOp.mult)

            # ---- two graph-conv layers ----
            call_no = 0
            for L in range(2):
                tab_h = tab1 if L == 0 else tab2f
                for g in range(n_groups):
                    bs = list(range(g * SG, min((g + 1) * SG, NB)))
                    my_calls = [cl for cl in calls if cl[3] in bs]
                    # one PSUM bank per block in the supergroup
                    pair = {}
                    for k, b in enumerate(bs):
                        pair[b] = pp_s.tile([P, 256], F32, space="PSUM",
                                            tag="ps", name=f"ps{L}_{g}_{k}")
                    first_c = {b: None for b in bs}
                    last_c = {}
                    for (c0, w, r, b) in my_calls:
                        if first_c[b] is None:
                            first_c[b] = c0
                        last_c[b] = c0 + w - 1
                    for (c0, w, r, b) in my_calls:
                        ci = call_no % NQ
                        gt = g_tiles[call_no % GB]
                        call_no += 1
                        nc.gpsimd.dma_gather(
                            out_ap=gt[:, :w, :],
                            in_ap=tab_h[r * RSZ:, :],
                            idxs_ap=gidx_s[:, c0 * 8:(c0 + w) * 8],
                            num_idxs=w * P, num_idxs_reg=nidx_regs.get(w, w * P),
                            elem_size=P, queue_num=ci)
                        psum = pair[b]
                        mt = mpool.tile([P, WMAX, 256], F16, tag="m",
                                        name=f"m{L}_{c0}")
                        if L == 1:
                            nc.sync.dma_start(out=mt[:, :w, :],
                                              in_=masks_d[:, c0:c0 + w, :])
                        for j in range(w):
                            c = c0 + j
                            if L == 0:
                                if c % ACTF == ACTF - 1:
                                    eqm = mpool.tile([P, P], F16, tag="eq",
                                                     name=f"eq{L}_{c}")
                                    nc.vector.tensor_scalar(
                                        out=eqm, in0=iota[:, :],
                                        scalar1=lr_s[:, c:c + 1], scalar2=None,
                                        op0=AluOp.is_equal)
                                    nc.scalar.mul(mt[:, j, 0:P], eqm,
                                                  wr_s[:, c:c + 1])
                                    nc.scalar.mul(mt[:, j, P:256], eqm,
                                                  wi_s[:, c:c + 1])
                                else:
                                    nc.vector.tensor_scalar(
                                        out=mt[:, j, 0:P], in0=iota[:, :],
                                        scalar1=lr_s[:, c:c + 1],
                                        scalar2=wr_s[:, c:c + 1],
                                        op0=AluOp.is_equal, op1=AluOp.mult)
                                    nc.vector.tensor_scalar(
                                        out=mt[:, j, P:256], in0=iota[:, :],
                                        scalar1=lr_s[:, c:c + 1],
                                        scalar2=wi_s[:, c:c + 1],
                                        op0=AluOp.is_equal, op1=AluOp.mult)
                            nc.tensor.matmul(
                                psum[:, :],
                                lhsT=gt[:, j, :], rhs=mt[:, j, :],
                                start=(c == first_c[b]), stop=(c == last_c[b]),
                                skip_group_check=True)
                        if L == 0:
                            nc.sync.dma_start(out=masks_d[:, c0:c0 + w, :],
                                              in_=mt[:, :w, :])
                    # finalize blocks of this supergroup
                    for b in bs:
                        psum = pair[b]
                        p2c = p2pool.tile([P, 256], F16, tag="p2",
                                          name=f"p2_{L}_{b}")
                        nc.scalar.activation(out=p2c, in_=psum[:, :],
                                             func=Act.Copy)
                        psl = pp_l.tile([P, P], F32, space="PSUM", tag="pl",
                                        name=f"pl{L}_{b}")
                        nc.tensor.matmul(psl[:, :], lhsT=wa[L], rhs=p2c[:, 0:P],
                                         start=True, stop=False)
                        nc.tensor.matmul(psl[:, :], lhsT=wb[L],
                                         rhs=p2c[:, P:256],
                                         start=False, stop=True)
                        lout = lopool.tile([P, P], F16, tag="lo",
                                           name=f"lo{L}_{b}")
                        nc.scalar.activation(out=lout, in_=psl, func=Act.Relu,
                                             bias=brow[L][:, 0:1])
                        nv = P if b < NB - 1 else cfg.NV_LAST
                        if L == 0:
                            pst = pp_x.tile([P, P], F16, space="PSUM",
                                            tag="px", name=f"px{b}")
                            nc.tensor.transpose(pst[:, :], lout[:, :],
                                                ident[:, :])
                            tblw = twpool.tile([P, P], F16, tag="tw",
                                               name=f"tw{b}")
                            nc.vector.tensor_copy(out=tblw, in_=pst)
                            nc.sync.dma_start(
                                out=tab2in[b * P:b * P + nv, :],
                                in_=tblw[:nv, :])
                        else:
                            pso = pp_x.tile([P, P], F32, space="PSUM",
                                            tag="px", name=f"pxo{b}")
                            nc.tensor.matmul(pso[:O, :], lhsT=w3s[:, :],
                                             rhs=lout[:, :], start=True,
                                             stop=True)
                            osb = topool.tile([O, P], F32, tag="to",
                                              name=f"to{b}")
                            nc.scalar.activation(out=osb, in_=pso[:O, :],
                                                 func=Act.Identity,
                                                 bias=b3c[:, 0:1])
                            nc.sync.dma_start(out=out_t[:, b * P:b * P + nv],
                                              in_=osb[:, :nv])
                if L == 0:
                    nc.gpsimd.collective_compute(
                        "AllGather", AluOp.bypass,
                        replica_groups=[list(range(cfg.CORES))],
                        ins=[tab2in.ap().opt()],
                        outs=[tab2f.ap().opt()],
                    )
    nc.compile()
    return nc


_CACHE = {}


def _get_nc(cfg, meta):
    key = (cfg.N, cfg.E, cfg.CORES, cfg.SG,
           tuple(c for call in meta["calls"] for c in call))
    if key not in _CACHE:
        _CACHE[key] = build_nc(cfg, meta)
    return _CACHE[key]


def run(cfg, inputs, trace=False):
    from concourse.bass_utils import run_bass_kernel_spmd

    in_maps, meta = host_prep(
        cfg,
        np.asarray(inputs["real_feature"], np.float32),
        np.asarray(inputs["imag_feature"], np.float32),
        np.asarray(inputs["edge_weight_sym"], np.float32),
        np.float32(inputs["exp_weight_q"]),
        np.asarray(inputs["edge_entropy"], np.float32),
        np.asarray(inputs["edge_cluster_coefficient"], np.float32),
        np.asarray(inputs["W1"], np.float32), np.asarray(inputs["b1"], np.float32),
        np.asarray(inputs["W2"], np.float32), np.asarray(inputs["b2"], np.float32),
        np.asarray(inputs["W3"], np.float32), np.asarray(inputs["b3"], np.float32),
        np.asarray(inputs["row"]).astype(np.int64),
        np.asarray(inputs["col"]).astype(np.int64),
    )
    nc = _get_nc(cfg, meta)
    res = run_bass_kernel_spmd(nc, in_maps, list(range(cfg.CORES)), trace=trace)
    out = np.empty((cfg.N, 16), np.float32)
    for c in range(cfg.CORES):
        out[c * cfg.NPC:(c + 1) * cfg.NPC, :] = res.results[c]["out_t"].T
    return out, res


def kernel(**inputs) -> np.ndarray:
    cfg = Cfg(100000, 1000000, cores=8)
    out, _ = run(cfg, inputs, trace=False)
    return out
